# revision 29
# baseline (speedup 1.0000x reference)
"""GCN layer (GraphConv norm='both' + BatchNorm1d + ReLU + residual) on 8 trn2 cores.

Sharding: nodes are split into 8 contiguous shards (one per core).  Each core
aggregates the in-edges of its own nodes (1D graph partitioning by dst).
BatchNorm batch statistics are combined with an 8-core AllReduce.

The SpMM gather uses the batched SWDGE gather instruction (InstDMAGatherAnt):
one instruction fetches up to 4096 rows of 256 B, and calls are spread
round-robin over the 4 SWDGE queues, which execute concurrently (~2 ns/row vs
~9.4 ns/row on one queue, vs ~1.1 us per 128 rows for per-column indirect
DMA).  The instruction's int16 index limit is handled with per-group windowed
gather tables: for each gather group (<= 2 x 32 slot columns) the host dedups
the referenced src rows into a window of < 32k rows (halo/ghost rows, ~2
copies per node), and slot indices become window-local.  Padding slots point
at a per-window zero row.  BatchNorm sums ride the ACT accumulator
(accum_out) on the existing h2T evictions.

Device-side pipeline per core (all cores run identical IR):
  1. dma_gather of y rows (y = x * norm_src) for each group of slot columns
  2. per-bucket strided tensor_reduce -> agg[128 nodes, 64]
  3. norm_dst scale (ACT, per-partition scalar), PE transpose, h2^T = W^T agg^T
  4. BN stats (sum, sum-of-squares) reduced on-core then AllReduce'd
  5. out = x + relu(h2 * A + B) fused via ACT scale/bias in transposed layout
"""

import os
import sys

import numpy as np

for _p in ("/opt/trn_rl_repo", "/root/.axon_site/_ro/trn_rl_repo"):
    if os.path.isdir(_p) and _p not in sys.path:
        sys.path.append(_p)

import concourse.tile as tile
from concourse import bacc, mybir
from concourse.masks import make_identity

N_CORES = 8
D = 64
P = 128
EPS = 1e-5
GCOLS = 32  # max slot columns per gather group (128*32 = 4096 idxs per call)

f32 = mybir.dt.float32
i16 = mybir.dt.int16


# ----------------------------------------------------------------------------
# Host-side graph partitioning
# ----------------------------------------------------------------------------
def _build_plan(src, dst, n_nodes):
    """Partition nodes across cores, bucket them by in-degree, build padded
    gather-slot schedules shared by all cores (IR shape must be identical)."""
    E = src.shape[0]
    deg_out = np.bincount(src, minlength=n_nodes)
    deg_in = np.bincount(dst, minlength=n_nodes)
    ns = np.maximum(deg_out, 1).astype(np.float32) ** -0.5
    nd = np.maximum(deg_in, 1).astype(np.float32) ** -0.5

    # CSR of in-edges (sorted by dst)
    order = np.argsort(dst, kind="stable")
    csr_src = src[order]
    starts = np.zeros(n_nodes + 1, dtype=np.int64)
    np.cumsum(deg_in, out=starts[1:])

    npc = (n_nodes + N_CORES - 1) // N_CORES  # nodes per core
    C = (npc + P - 1) // P  # chunks (buckets) per core
    rows = C * P

    # per-core node permutation, sorted by in-degree descending
    perms = np.full((N_CORES, rows), -1, dtype=np.int64)
    degs = np.zeros((N_CORES, rows), dtype=np.int64)
    for c in range(N_CORES):
        lo, hi = c * npc, min((c + 1) * npc, n_nodes)
        nodes = np.arange(lo, hi)
        o = np.argsort(-deg_in[nodes], kind="stable")
        perms[c, : hi - lo] = nodes[o]
        degs[c, : hi - lo] = deg_in[nodes[o]]

    # shared bucket max-degree schedule
    K = degs.reshape(N_CORES, C, P).max(axis=(0, 2)).astype(np.int64)
    off = np.zeros(C + 1, dtype=np.int64)
    np.cumsum(K, out=off[1:])
    M = int(off[-1])

    # gather groups: consecutive buckets, <= GCOLS slot columns each
    groups = []  # (b0, b1) bucket ranges, all with K>0
    b = 0
    while b < C and K[b] > 0:
        b1 = b
        cols = 0
        while b1 < C and K[b1] > 0 and (b1 == b or cols + K[b1] <= GCOLS):
            cols += int(K[b1])
            b1 += 1
        groups.append((b, b1))
        b = b1
    zero_buckets = [j for j in range(C) if K[j] == 0]

    # per-core slot -> absolute node id (n_nodes = padding sentinel)
    idx_all = np.full((N_CORES, P, M), n_nodes, dtype=np.int32)
    nd_all = np.ones((N_CORES, P, C), dtype=np.float32)
    for c in range(N_CORES):
        for j in range(C):
            kj = int(K[j])
            if kj == 0:
                continue
            nodes_b = perms[c, j * P : (j + 1) * P]
            degs_b = degs[c, j * P : (j + 1) * P]
            ks = np.arange(kj)
            pos = starts[np.maximum(nodes_b, 0)][:, None] + ks[None, :]
            valid = (ks[None, :] < degs_b[:, None]) & (nodes_b[:, None] >= 0)
            blk = np.where(valid, csr_src[np.minimum(pos, E - 1)], n_nodes)
            idx_all[c, :, off[j] : off[j + 1]] = blk.astype(np.int32)
        real = perms[c] >= 0
        nd_all[c, :, :] = np.where(
            real, nd[np.maximum(perms[c], 0)], 1.0
        ).reshape(C, P).T

    return dict(
        ns=ns, nd=nd, perms=perms, K=K, off=off, M=M, C=C, rows=rows,
        groups=groups, zero_buckets=zero_buckets, idx=idx_all, nd_dev=nd_all,
    )


def _build_windows(plan, n_nodes):
    """Per-(core, group) windowed gather tables: dedup the referenced node
    rows into windows of < 32k rows so slot indices fit int16.  Window row 0
    is a zero row (padding-slot target).  Window starts/sizes are shared
    across cores (SPMD: identical IR)."""
    idx_all, off, groups = plan["idx"], plan["off"], plan["groups"]
    n_groups = len(groups)

    win_nodes = [[None] * n_groups for _ in range(N_CORES)]
    lidx = [[None] * n_groups for _ in range(N_CORES)]
    sizes = np.zeros((N_CORES, n_groups), dtype=np.int64)
    for c in range(N_CORES):
        for g, (b0, b1) in enumerate(groups):
            slots = idx_all[c][:, off[b0] : off[b1]]  # [P, cols] absolute ids
            uniq = np.unique(slots)
            uniq = uniq[uniq < n_nodes]  # drop the padding sentinel
            loc = np.searchsorted(uniq, slots)
            hit = (loc < len(uniq)) & (slots < n_nodes)
            hit &= np.where(hit, uniq[np.minimum(loc, len(uniq) - 1)] == slots, False)
            win_nodes[c][g] = uniq
            lidx[c][g] = np.where(hit, loc + 1, 0).astype(np.int16)
            sizes[c, g] = 1 + len(uniq)
            assert sizes[c, g] <= 32768, f"window {g} too big: {sizes[c, g]}"

    Rg = sizes.max(axis=0)  # shared per-group window capacity
    Wstart = np.zeros(n_groups + 1, dtype=np.int64)
    np.cumsum(Rg, out=Wstart[1:])
    return dict(win_nodes=win_nodes, lidx=lidx, Rg=Rg, Wstart=Wstart,
                R=int(Wstart[-1]), n_groups=n_groups)


def _pack_idx(plan, wins):
    """int16 index tensor [P, 8*M]: group g occupies free range
    [8*off[b0], 8*off[b1]); position j = c_local*128 + p holds lidx[p,c_local]
    at [j%16, base + j//16], replicated x8 across 16-partition groups."""
    off, groups, M = plan["off"], plan["groups"], plan["M"]
    out = np.zeros((N_CORES, P, 8 * M), dtype=np.int16)
    for c in range(N_CORES):
        for g, (b0, b1) in enumerate(groups):
            li = wins["lidx"][c][g]  # [P, cols]
            cols = li.shape[1]
            flat = li.T.reshape(-1)  # position j = c_local*128 + p
            wrapped = flat.reshape(8 * cols, 16).T  # [16, 8*cols]
            base = 8 * int(off[b0])
            for r in range(8):
                out[c, r * 16 : (r + 1) * 16, base : base + 8 * cols] = wrapped
    return out


# ----------------------------------------------------------------------------
# Device IR
# ----------------------------------------------------------------------------
def _build_bass(n_nodes, C, M, K, off, groups, zero_buckets, Rg, Wstart, R, n_real):
    nc = bacc.Bacc(
        "TRN2",
        target_bir_lowering=False,
        debug=False,
        enable_asserts=False,
        num_devices=N_CORES,
        num_swdge_queues=4,
    )
    t_d = nc.dram_tensor("t", [R, D], f32, kind="ExternalInput").ap()
    idx_d = nc.dram_tensor("idx", [P, 8 * M], i16, kind="ExternalInput").ap()
    nd_d = nc.dram_tensor("nd", [P, C], f32, kind="ExternalInput").ap()
    xp_d = nc.dram_tensor("xp", [P, C * D], f32, kind="ExternalInput").ap()
    w_d = nc.dram_tensor("w", [D, D], f32, kind="ExternalInput").ap()
    g_d = nc.dram_tensor("gam", [D, 1], f32, kind="ExternalInput").ap()
    b_d = nc.dram_tensor("bet", [D, 1], f32, kind="ExternalInput").ap()
    out_d = nc.dram_tensor("out", [P, C * D], f32, kind="ExternalOutput").ap()
    if os.environ.get("KDBG"):
        agg_d = nc.dram_tensor("agg_dbg", [P, C * D], f32, kind="ExternalOutput").ap()
        h2t_d = nc.dram_tensor("h2t_dbg", [D, C * P], f32, kind="ExternalOutput").ap()
        st_d = nc.dram_tensor("st_dbg", [D, 2], f32, kind="ExternalOutput").ap()

    FP = C * P  # free size of h2T (nodes per core incl. padding)
    inv_n = 1.0 / float(n_real)

    with tile.TileContext(nc) as tc:
        with (
            tc.tile_pool(name="persist", bufs=1) as pp,
            tc.tile_pool(name="gather", bufs=8) as gp,
            tc.tile_pool(name="idxp", bufs=2) as ip,
            tc.tile_pool(name="work", bufs=3) as wp,
            tc.tile_pool(name="psum", bufs=2, space="PSUM") as psp,
            tc.tile_pool(name="dram", bufs=1, space="DRAM") as dp,
        ):
            # ---- constant loads
            NIDXT = 4  # idx upload chunks (finer gather gating)
            g_order = sorted(range(len(groups)),
                             key=lambda g: -(groups[g][1] - groups[g][0]))
            gpc = (len(groups) + NIDXT - 1) // NIDXT
            chunk_of = {}
            chunk_range = {}
            for q in range(NIDXT):
                gs = groups[q * gpc : (q + 1) * gpc]
                if not gs:
                    continue
                chunk_range[q] = (8 * int(off[gs[0][0]]), 8 * int(off[gs[-1][1]]))
                for gi in range(q * gpc, min((q + 1) * gpc, len(groups))):
                    chunk_of[gi] = q
            maxw = max(ce - cb for cb, ce in chunk_range.values())
            idx_tiles = {}
            for g in g_order:  # load chunks in first-use order
                q = chunk_of[g]
                if q in idx_tiles:
                    continue
                cb, ce = chunk_range[q]
                tq = ip.tile([P, maxw], i16, tag="idxc")
                nc.sync.dma_start(out=tq[:, : ce - cb], in_=idx_d[:, cb:ce])
                idx_tiles[q] = (tq, cb)
            nd_sb = pp.tile([P, C], f32, tag="nd")
            x_sb = pp.tile([P, C * D], f32, tag="x")
            w_sb = pp.tile([D, D], f32, tag="w")
            gam_sb = pp.tile([D, 1], f32, tag="gam")
            bet_sb = pp.tile([D, 1], f32, tag="bet")
            ident = pp.tile([P, P], f32, tag="ident")
            nc.sync.dma_start(out=nd_sb[:], in_=nd_d[:])
            nc.sync.dma_start(out=x_sb[:], in_=xp_d[:])
            nc.sync.dma_start(out=w_sb[:], in_=w_d[:])
            nc.sync.dma_start(out=gam_sb[:], in_=g_d[:])
            nc.sync.dma_start(out=bet_sb[:], in_=b_d[:])
            make_identity(nc, ident[:])

            agg = pp.tile([P, C * D], f32, tag="agg")
            h2T = pp.tile([D, FP], f32, tag="h2T")
            s1b = pp.tile([D, C], f32, tag="s1b")

            # ---- phase A+B: gather -> reduce -> scale/transpose/matmul
            def tail_chunk(j):
                """agg chunk j -> h2T chunk j"""
                aggS = wp.tile([P, D], f32, tag="aggS")
                nc.scalar.activation(
                    out=aggS[:], in_=agg[:, j * D : (j + 1) * D],
                    func=mybir.ActivationFunctionType.Copy,
                    scale=nd_sb[:, j : j + 1],
                )
                aggT_p = psp.tile([D, P], f32, tag="aggT_p")
                nc.tensor.transpose(out=aggT_p[:], in_=aggS[:], identity=ident[:])
                aggT_sb = wp.tile([D, P], f32, tag="aggT_sb")
                nc.scalar.activation(
                    out=aggT_sb[:], in_=aggT_p[:],
                    func=mybir.ActivationFunctionType.Copy,
                )
                h2T_p = psp.tile([D, P], f32, tag="h2T_p")
                nc.tensor.matmul(
                    out=h2T_p[:], lhsT=w_sb[:], rhs=aggT_sb[:],
                    start=True, stop=True,
                )
                nc.scalar.activation(
                    out=h2T[:, j * P : (j + 1) * P], in_=h2T_p[:],
                    func=mybir.ActivationFunctionType.Copy,
                    accum_out=s1b[:, j : j + 1],
                )

            for j in zero_buckets:
                nc.vector.memset(agg[:, j * D : (j + 1) * D], 0.0)
                tail_chunk(j)

            # batched SWDGE gathers: per group, sub-calls of <= GCOLS
            # columns, round-robin over the 4 SWDGE queues (parallel Q7)
            gmax = max(int(off[b1] - off[b0]) for b0, b1 in groups)
            ncall = 0
            for g in g_order:
                b0, b1 = groups[g]
                cols = int(off[b1] - off[b0])
                gt = gp.tile([P, gmax * D], f32, tag="g")
                nsub = (cols + GCOLS - 1) // GCOLS
                sub = (cols + nsub - 1) // nsub
                for s0 in range(0, cols, sub):
                    s1 = min(s0 + sub, cols)
                    jb = 8 * (int(off[b0]) + s0)
                    je = 8 * (int(off[b0]) + s1)
                    nc.gpsimd.dma_gather(
                        out_ap=gt[:, s0 * D : s1 * D].rearrange(
                            "p (c d) -> p c d", d=D
                        ),
                        in_ap=t_d[int(Wstart[g]) : int(Wstart[g]) + int(Rg[g])],
                        idxs_ap=idx_tiles[chunk_of[g]][0][
                            :, jb - idx_tiles[chunk_of[g]][1] : je - idx_tiles[chunk_of[g]][1]
                        ],
                        num_idxs=P * (s1 - s0),
                        num_idxs_reg=P * (s1 - s0),
                        elem_size=D,
                        single_packet=False,
                        queue_num=ncall % 4,
                    )
                    ncall += 1
                for j in range(b0, b1):
                    kj = int(K[j])
                    lo = (int(off[j]) - int(off[b0])) * D
                    src3d = gt[:, lo : lo + kj * D].rearrange(
                        "p (k d) -> p d k", d=D
                    )
                    nc.vector.tensor_reduce(
                        out=agg[:, j * D : (j + 1) * D],
                        in_=src3d,
                        axis=mybir.AxisListType.X,
                        op=mybir.AluOpType.add,
                    )
                    tail_chunk(j)

            # ---- phase C: BN statistics + AllReduce
            stats = pp.tile([D, 2], f32, tag="stats")
            piece = 8 * D  # small sink keeps SBUF headroom
            n_pieces = (FP + piece - 1) // piece
            s2parts = pp.tile([D, n_pieces], f32, tag="s2parts")
            sqsink = pp.tile([D, piece], f32, tag="sqsink")
            for q in range(n_pieces):
                lo, hi = q * piece, min((q + 1) * piece, FP)
                nc.scalar.activation(
                    out=sqsink[:, : hi - lo], in_=h2T[:, lo:hi],
                    func=mybir.ActivationFunctionType.Square,
                    accum_out=s2parts[:, q : q + 1],
                )
            nc.vector.tensor_reduce(
                out=stats[:, 0:1], in_=s1b[:],
                axis=mybir.AxisListType.X, op=mybir.AluOpType.add,
            )
            nc.vector.tensor_reduce(
                out=stats[:, 1:2], in_=s2parts[:],
                axis=mybir.AxisListType.X, op=mybir.AluOpType.add,
            )
            if os.environ.get("KDBG"):
                nc.sync.dma_start(out=agg_d[:], in_=agg[:])
                nc.sync.dma_start(out=h2t_d[:], in_=h2T[:])
                nc.sync.dma_start(out=st_d[:], in_=stats[:])

            ar_in = dp.tile([D, 2], f32)
            ar_out = dp.tile([D, 2], f32)
            nc.sync.dma_start(out=ar_in[:], in_=stats[:])
            nc.gpsimd.collective_compute(
                "AllReduce",
                mybir.AluOpType.add,
                replica_groups=[list(range(N_CORES))],
                ins=[ar_in.opt()],
                outs=[ar_out.opt()],
            )
            arr = pp.tile([D, 2], f32, tag="arr")
            nc.sync.dma_start(out=arr[:], in_=ar_out[:])

            mean = pp.tile([D, 1], f32, tag="mean")
            var = pp.tile([D, 1], f32, tag="var")
            tmp = pp.tile([D, 1], f32, tag="tmp")
            A = pp.tile([D, 1], f32, tag="A")
            B = pp.tile([D, 1], f32, tag="B")
            epsT = pp.tile([D, 1], f32, tag="epsT")
            nc.vector.memset(epsT[:], EPS)
            nc.vector.tensor_scalar_mul(mean[:], arr[:, 0:1], inv_n)
            nc.vector.tensor_scalar_mul(var[:], arr[:, 1:2], inv_n)
            nc.vector.tensor_mul(tmp[:], mean[:], mean[:])
            nc.vector.tensor_sub(var[:], var[:], tmp[:])
            # std = sqrt(var + eps); inv = 1/std
            nc.scalar.activation(
                out=tmp[:], in_=var[:],
                func=mybir.ActivationFunctionType.Sqrt, bias=epsT[:],
            )
            nc.vector.reciprocal(var[:], tmp[:])  # var now holds 1/std
            nc.vector.tensor_mul(A[:], var[:], gam_sb[:])
            nc.vector.tensor_mul(tmp[:], mean[:], A[:])
            nc.vector.tensor_sub(B[:], bet_sb[:], tmp[:])

            # ---- phase D: relu(h2*A+B) + x, back to node-major layout
            for j0 in range(0, C, 8):
                j1 = min(j0 + 8, C)
                rT = wp.tile([D, 8 * P], f32, tag="rT")
                nc.scalar.activation(
                    out=rT[:, : (j1 - j0) * P], in_=h2T[:, j0 * P : j1 * P],
                    func=mybir.ActivationFunctionType.Relu,
                    scale=A[:], bias=B[:],
                )
                r_p = psp.tile([P, 8 * D], f32, tag="r_p")
                for j in range(j0, j1):
                    nc.tensor.transpose(
                        out=r_p[:, (j - j0) * D : (j - j0 + 1) * D],
                        in_=rT[:, (j - j0) * P : (j - j0 + 1) * P],
                        identity=ident[:D, :D],
                    )
                nc.vector.tensor_add(
                    agg[:, j0 * D : j1 * D], r_p[:, : (j1 - j0) * D],
                    x_sb[:, j0 * D : j1 * D],
                )
                if j1 % 16 == 0 or j1 == C:
                    ja = (j1 - 1) // 16 * 16
                    nc.sync.dma_start(
                        out=out_d[:, ja * D : j1 * D],
                        in_=agg[:, ja * D : j1 * D],
                    )

    nc.compile()
    return nc


# ----------------------------------------------------------------------------
# Entry point
# ----------------------------------------------------------------------------
def kernel(x, src, dst, W, b, gamma, beta):
    from concourse.bass_utils import run_bass_kernel_spmd

    x = np.asarray(x, dtype=np.float32)
    src = np.asarray(src, dtype=np.int32)
    dst = np.asarray(dst, dtype=np.int32)
    W = np.asarray(W, dtype=np.float32)
    gamma = np.asarray(gamma, dtype=np.float32)
    beta = np.asarray(beta, dtype=np.float32)
    n_nodes = x.shape[0]
    assert x.shape[1] == D

    plan = _build_plan(src, dst, n_nodes)
    wins = _build_windows(plan, n_nodes)
    idx_packed = _pack_idx(plan, wins)
    C, M, rows = plan["C"], plan["M"], plan["rows"]
    R, Rg, Wstart = wins["R"], wins["Rg"], wins["Wstart"]

    # gather tables: y = x * norm_src rows, windowed + deduped per group
    y = x * plan["ns"][:, None]

    in_maps = []
    for c in range(N_CORES):
        t_c = np.zeros((R, D), dtype=np.float32)
        for g in range(wins["n_groups"]):
            wn = wins["win_nodes"][c][g]
            s = int(Wstart[g])
            t_c[s + 1 : s + 1 + len(wn)] = y[wn]
        perm = plan["perms"][c]
        xp = np.zeros((rows, D), dtype=np.float32)
        real = perm >= 0
        xp[real] = x[perm[real]]
        # [rows, D] -> [P, C*D] with row j*128+p at (p, j*64:(j+1)*64)
        xp_dev = np.ascontiguousarray(
            xp.reshape(C, P, D).transpose(1, 0, 2).reshape(P, C * D)
        )
        in_maps.append(
            dict(
                t=t_c,
                idx=np.ascontiguousarray(idx_packed[c]),
                nd=np.ascontiguousarray(plan["nd_dev"][c]),
                xp=xp_dev,
                w=W,
                gam=gamma.reshape(D, 1),
                bet=beta.reshape(D, 1),
            )
        )

    nc = _build_bass(
        n_nodes, C, M, plan["K"], plan["off"], plan["groups"],
        plan["zero_buckets"], Rg, Wstart, R, n_nodes,
    )
    res = run_bass_kernel_spmd(nc, in_maps, core_ids=list(range(N_CORES)))
    kernel.last_results = res

    out = np.empty_like(x)
    for c in range(N_CORES):
        o = res.results[c]["out"]  # [P, C*D]
        o_rows = o.reshape(P, C, D).transpose(1, 0, 2).reshape(rows, D)
        perm = plan["perms"][c]
        real = perm >= 0
        out[perm[real]] = o_rows[real]
    return out


# revision 30
# speedup vs baseline: 1.0910x; 1.0910x over previous
"""GCN layer (GraphConv norm='both' + BatchNorm1d + ReLU + residual) on 8 trn2 cores.

Sharding: nodes are split into 8 contiguous shards (one per core).  Each core
aggregates the in-edges of its own nodes (1D graph partitioning by dst).
BatchNorm batch statistics are combined with an 8-core AllReduce.

The SpMM gather uses the batched SWDGE gather instruction (InstDMAGatherAnt):
one instruction fetches up to 4096 rows of 256 B, and calls are spread
round-robin over the 4 SWDGE queues, which execute concurrently (~2 ns/row vs
~9.4 ns/row on one queue, vs ~1.1 us per 128 rows for per-column indirect
DMA).  The instruction's int16 index limit is handled with per-group windowed
gather tables: for each gather group (<= 2 x 32 slot columns) the host dedups
the referenced src rows into a window of < 32k rows (halo/ghost rows, ~2
copies per node), and slot indices become window-local.  Padding slots point
at a per-window zero row.  BatchNorm sums ride the ACT accumulator
(accum_out) on the existing h2T evictions.

Device-side pipeline per core (all cores run identical IR):
  1. dma_gather of y rows (y = x * norm_src) for each group of slot columns
  2. per-bucket strided tensor_reduce -> agg[128 nodes, 64]
  3. norm_dst scale (ACT, per-partition scalar), PE transpose, h2^T = W^T agg^T
  4. BN stats (sum, sum-of-squares) reduced on-core then AllReduce'd
  5. out = x + relu(h2 * A + B) fused via ACT scale/bias in transposed layout
"""

import os
import sys

import numpy as np

for _p in ("/opt/trn_rl_repo", "/root/.axon_site/_ro/trn_rl_repo"):
    if os.path.isdir(_p) and _p not in sys.path:
        sys.path.append(_p)

import concourse.tile as tile
from concourse import bacc, mybir
from concourse.masks import make_identity

N_CORES = 8
D = 64
P = 128
EPS = 1e-5
GCOLS = 32  # max slot columns per gather group (128*32 = 4096 idxs per call)

f32 = mybir.dt.float32
i16 = mybir.dt.int16


# ----------------------------------------------------------------------------
# Host-side graph partitioning
# ----------------------------------------------------------------------------
def _build_plan(src, dst, n_nodes):
    """Partition nodes across cores, bucket them by in-degree, build padded
    gather-slot schedules shared by all cores (IR shape must be identical)."""
    E = src.shape[0]
    deg_out = np.bincount(src, minlength=n_nodes)
    deg_in = np.bincount(dst, minlength=n_nodes)
    ns = np.maximum(deg_out, 1).astype(np.float32) ** -0.5
    nd = np.maximum(deg_in, 1).astype(np.float32) ** -0.5

    # CSR of in-edges (sorted by dst)
    order = np.argsort(dst, kind="stable")
    csr_src = src[order]
    starts = np.zeros(n_nodes + 1, dtype=np.int64)
    np.cumsum(deg_in, out=starts[1:])

    npc = (n_nodes + N_CORES - 1) // N_CORES  # nodes per core
    C = (npc + P - 1) // P  # chunks (buckets) per core
    rows = C * P

    # per-core node permutation, sorted by in-degree descending
    perms = np.full((N_CORES, rows), -1, dtype=np.int64)
    degs = np.zeros((N_CORES, rows), dtype=np.int64)
    for c in range(N_CORES):
        lo, hi = c * npc, min((c + 1) * npc, n_nodes)
        nodes = np.arange(lo, hi)
        o = np.argsort(-deg_in[nodes], kind="stable")
        perms[c, : hi - lo] = nodes[o]
        degs[c, : hi - lo] = deg_in[nodes[o]]

    # shared bucket max-degree schedule
    K = degs.reshape(N_CORES, C, P).max(axis=(0, 2)).astype(np.int64)
    off = np.zeros(C + 1, dtype=np.int64)
    np.cumsum(K, out=off[1:])
    M = int(off[-1])

    # gather groups: consecutive buckets, <= GCOLS slot columns each
    groups = []  # (b0, b1) bucket ranges, all with K>0
    b = 0
    while b < C and K[b] > 0:
        b1 = b
        cols = 0
        while b1 < C and K[b1] > 0 and (b1 == b or cols + K[b1] <= GCOLS):
            cols += int(K[b1])
            b1 += 1
        groups.append((b, b1))
        b = b1
    zero_buckets = [j for j in range(C) if K[j] == 0]

    # per-core slot -> absolute node id (n_nodes = padding sentinel)
    idx_all = np.full((N_CORES, P, M), n_nodes, dtype=np.int32)
    nd_all = np.ones((N_CORES, P, C), dtype=np.float32)
    for c in range(N_CORES):
        for j in range(C):
            kj = int(K[j])
            if kj == 0:
                continue
            nodes_b = perms[c, j * P : (j + 1) * P]
            degs_b = degs[c, j * P : (j + 1) * P]
            ks = np.arange(kj)
            pos = starts[np.maximum(nodes_b, 0)][:, None] + ks[None, :]
            valid = (ks[None, :] < degs_b[:, None]) & (nodes_b[:, None] >= 0)
            blk = np.where(valid, csr_src[np.minimum(pos, E - 1)], n_nodes)
            idx_all[c, :, off[j] : off[j + 1]] = blk.astype(np.int32)
        real = perms[c] >= 0
        nd_all[c, :, :] = np.where(
            real, nd[np.maximum(perms[c], 0)], 1.0
        ).reshape(C, P).T

    return dict(
        ns=ns, nd=nd, perms=perms, K=K, off=off, M=M, C=C, rows=rows,
        groups=groups, zero_buckets=zero_buckets, idx=idx_all, nd_dev=nd_all,
    )


def _build_windows(plan, n_nodes):
    """Per-(core, group) windowed gather tables: dedup the referenced node
    rows into windows of < 32k rows so slot indices fit int16.  Window row 0
    is a zero row (padding-slot target).  Window starts/sizes are shared
    across cores (SPMD: identical IR)."""
    idx_all, off, groups = plan["idx"], plan["off"], plan["groups"]
    n_groups = len(groups)

    win_nodes = [[None] * n_groups for _ in range(N_CORES)]
    lidx = [[None] * n_groups for _ in range(N_CORES)]
    sizes = np.zeros((N_CORES, n_groups), dtype=np.int64)
    for c in range(N_CORES):
        for g, (b0, b1) in enumerate(groups):
            slots = idx_all[c][:, off[b0] : off[b1]]  # [P, cols] absolute ids
            uniq = np.unique(slots)
            uniq = uniq[uniq < n_nodes]  # drop the padding sentinel
            loc = np.searchsorted(uniq, slots)
            hit = (loc < len(uniq)) & (slots < n_nodes)
            hit &= np.where(hit, uniq[np.minimum(loc, len(uniq) - 1)] == slots, False)
            win_nodes[c][g] = uniq
            lidx[c][g] = np.where(hit, loc + 1, 0).astype(np.int16)
            sizes[c, g] = 1 + len(uniq)
            assert sizes[c, g] <= 32768, f"window {g} too big: {sizes[c, g]}"

    Rg = sizes.max(axis=0)  # shared per-group window capacity
    Wstart = np.zeros(n_groups + 1, dtype=np.int64)
    np.cumsum(Rg, out=Wstart[1:])
    return dict(win_nodes=win_nodes, lidx=lidx, Rg=Rg, Wstart=Wstart,
                R=int(Wstart[-1]), n_groups=n_groups)


def _pack_idx(plan, wins):
    """int16 index tensor [P, 8*M]: group g occupies free range
    [8*off[b0], 8*off[b1]); position j = c_local*128 + p holds lidx[p,c_local]
    at [j%16, base + j//16], replicated x8 across 16-partition groups."""
    off, groups, M = plan["off"], plan["groups"], plan["M"]
    out = np.zeros((N_CORES, P, 8 * M), dtype=np.int16)
    for c in range(N_CORES):
        for g, (b0, b1) in enumerate(groups):
            li = wins["lidx"][c][g]  # [P, cols]
            cols = li.shape[1]
            flat = li.T.reshape(-1)  # position j = c_local*128 + p
            wrapped = flat.reshape(8 * cols, 16).T  # [16, 8*cols]
            base = 8 * int(off[b0])
            for r in range(8):
                out[c, r * 16 : (r + 1) * 16, base : base + 8 * cols] = wrapped
    return out


# ----------------------------------------------------------------------------
# Device IR
# ----------------------------------------------------------------------------
def _build_bass(n_nodes, C, M, K, off, groups, zero_buckets, Rg, Wstart, R, n_real):
    nc = bacc.Bacc(
        "TRN2",
        target_bir_lowering=False,
        debug=False,
        enable_asserts=False,
        num_devices=N_CORES,
        num_swdge_queues=4,
    )
    t_d = nc.dram_tensor("t", [R, D], f32, kind="ExternalInput").ap()
    idx_d = nc.dram_tensor("idx", [P, 8 * M], i16, kind="ExternalInput").ap()
    nd_d = nc.dram_tensor("nd", [P, C], f32, kind="ExternalInput").ap()
    xp_d = nc.dram_tensor("xp", [P, C * D], f32, kind="ExternalInput").ap()
    w_d = nc.dram_tensor("w", [D, D], f32, kind="ExternalInput").ap()
    g_d = nc.dram_tensor("gam", [D, 1], f32, kind="ExternalInput").ap()
    b_d = nc.dram_tensor("bet", [D, 1], f32, kind="ExternalInput").ap()
    out_d = nc.dram_tensor("out", [P, C * D], f32, kind="ExternalOutput").ap()
    if os.environ.get("KDBG"):
        agg_d = nc.dram_tensor("agg_dbg", [P, C * D], f32, kind="ExternalOutput").ap()
        h2t_d = nc.dram_tensor("h2t_dbg", [D, C * P], f32, kind="ExternalOutput").ap()
        st_d = nc.dram_tensor("st_dbg", [D, 2], f32, kind="ExternalOutput").ap()

    FP = C * P  # free size of h2T (nodes per core incl. padding)
    inv_n = 1.0 / float(n_real)

    with tile.TileContext(nc) as tc:
        with (
            tc.tile_pool(name="persist", bufs=1) as pp,
            tc.tile_pool(name="gather", bufs=7) as gp,
            tc.tile_pool(name="work", bufs=3) as wp,
            tc.tile_pool(name="psum", bufs=2, space="PSUM") as psp,
            tc.tile_pool(name="dram", bufs=1, space="DRAM") as dp,
        ):
            # ---- constant loads
            NIDXT = 4  # idx upload chunks (finer gather gating)
            g_order = sorted(range(len(groups)),
                             key=lambda g: -(groups[g][1] - groups[g][0]))
            gpc = (len(groups) + NIDXT - 1) // NIDXT
            chunk_of = {}
            chunk_range = {}
            for q in range(NIDXT):
                gs = groups[q * gpc : (q + 1) * gpc]
                if not gs:
                    continue
                chunk_range[q] = (8 * int(off[gs[0][0]]), 8 * int(off[gs[-1][1]]))
                for gi in range(q * gpc, min((q + 1) * gpc, len(groups))):
                    chunk_of[gi] = q
            idx_tiles = {}
            for g in g_order:  # load chunks in first-use order
                q = chunk_of[g]
                if q in idx_tiles:
                    continue
                cb, ce = chunk_range[q]
                tq = pp.tile([P, ce - cb], i16, tag=f"idx{q}")
                nc.sync.dma_start(out=tq[:], in_=idx_d[:, cb:ce])
                idx_tiles[q] = (tq, cb)
            nd_sb = pp.tile([P, C], f32, tag="nd")
            x_sb = pp.tile([P, C * D], f32, tag="x")
            w_sb = pp.tile([D, D], f32, tag="w")
            gam_sb = pp.tile([D, 1], f32, tag="gam")
            bet_sb = pp.tile([D, 1], f32, tag="bet")
            ident = pp.tile([P, P], f32, tag="ident")
            nc.sync.dma_start(out=nd_sb[:], in_=nd_d[:])
            nc.sync.dma_start(out=x_sb[:], in_=xp_d[:])
            nc.sync.dma_start(out=w_sb[:], in_=w_d[:])
            nc.sync.dma_start(out=gam_sb[:], in_=g_d[:])
            nc.sync.dma_start(out=bet_sb[:], in_=b_d[:])
            make_identity(nc, ident[:])

            agg = pp.tile([P, C * D], f32, tag="agg")
            h2T = pp.tile([D, FP], f32, tag="h2T")
            s1b = pp.tile([D, C], f32, tag="s1b")

            # ---- phase A+B: gather -> reduce -> scale/transpose/matmul
            def tail_chunk(j):
                """agg chunk j -> h2T chunk j"""
                aggS = wp.tile([P, D], f32, tag="aggS")
                nc.scalar.activation(
                    out=aggS[:], in_=agg[:, j * D : (j + 1) * D],
                    func=mybir.ActivationFunctionType.Copy,
                    scale=nd_sb[:, j : j + 1],
                )
                aggT_p = psp.tile([D, P], f32, tag="aggT_p")
                nc.tensor.transpose(out=aggT_p[:], in_=aggS[:], identity=ident[:])
                aggT_sb = wp.tile([D, P], f32, tag="aggT_sb")
                nc.scalar.activation(
                    out=aggT_sb[:], in_=aggT_p[:],
                    func=mybir.ActivationFunctionType.Copy,
                )
                h2T_p = psp.tile([D, P], f32, tag="h2T_p")
                nc.tensor.matmul(
                    out=h2T_p[:], lhsT=w_sb[:], rhs=aggT_sb[:],
                    start=True, stop=True,
                )
                nc.scalar.activation(
                    out=h2T[:, j * P : (j + 1) * P], in_=h2T_p[:],
                    func=mybir.ActivationFunctionType.Copy,
                    accum_out=s1b[:, j : j + 1],
                )

            for j in zero_buckets:
                nc.vector.memset(agg[:, j * D : (j + 1) * D], 0.0)
                tail_chunk(j)

            # batched SWDGE gathers: per group, sub-calls of <= GCOLS
            # columns, round-robin over the 4 SWDGE queues (parallel Q7)
            gmax = max(int(off[b1] - off[b0]) for b0, b1 in groups)
            ncall = 0
            for g in g_order:
                b0, b1 = groups[g]
                cols = int(off[b1] - off[b0])
                gt = gp.tile([P, gmax * D], f32, tag="g")
                nsub = (cols + GCOLS - 1) // GCOLS
                sub = (cols + nsub - 1) // nsub
                for s0 in range(0, cols, sub):
                    s1 = min(s0 + sub, cols)
                    jb = 8 * (int(off[b0]) + s0)
                    je = 8 * (int(off[b0]) + s1)
                    nc.gpsimd.dma_gather(
                        out_ap=gt[:, s0 * D : s1 * D].rearrange(
                            "p (c d) -> p c d", d=D
                        ),
                        in_ap=t_d[int(Wstart[g]) : int(Wstart[g]) + int(Rg[g])],
                        idxs_ap=idx_tiles[chunk_of[g]][0][
                            :, jb - idx_tiles[chunk_of[g]][1] : je - idx_tiles[chunk_of[g]][1]
                        ],
                        num_idxs=P * (s1 - s0),
                        num_idxs_reg=P * (s1 - s0),
                        elem_size=D,
                        single_packet=False,
                        queue_num=ncall % 4,
                    )
                    ncall += 1
                for j in range(b0, b1):
                    kj = int(K[j])
                    lo = (int(off[j]) - int(off[b0])) * D
                    src3d = gt[:, lo : lo + kj * D].rearrange(
                        "p (k d) -> p d k", d=D
                    )
                    nc.vector.tensor_reduce(
                        out=agg[:, j * D : (j + 1) * D],
                        in_=src3d,
                        axis=mybir.AxisListType.X,
                        op=mybir.AluOpType.add,
                    )
                    tail_chunk(j)

            # ---- phase C: BN statistics + AllReduce
            stats = pp.tile([D, 2], f32, tag="stats")
            piece = 16 * D  # small sink keeps SBUF headroom
            n_pieces = (FP + piece - 1) // piece
            s2parts = pp.tile([D, n_pieces], f32, tag="s2parts")
            sqsink = pp.tile([D, piece], f32, tag="sqsink")
            for q in range(n_pieces):
                lo, hi = q * piece, min((q + 1) * piece, FP)
                nc.scalar.activation(
                    out=sqsink[:, : hi - lo], in_=h2T[:, lo:hi],
                    func=mybir.ActivationFunctionType.Square,
                    accum_out=s2parts[:, q : q + 1],
                )
            nc.vector.tensor_reduce(
                out=stats[:, 0:1], in_=s1b[:],
                axis=mybir.AxisListType.X, op=mybir.AluOpType.add,
            )
            nc.vector.tensor_reduce(
                out=stats[:, 1:2], in_=s2parts[:],
                axis=mybir.AxisListType.X, op=mybir.AluOpType.add,
            )
            if os.environ.get("KDBG"):
                nc.sync.dma_start(out=agg_d[:], in_=agg[:])
                nc.sync.dma_start(out=h2t_d[:], in_=h2T[:])
                nc.sync.dma_start(out=st_d[:], in_=stats[:])

            ar_in = dp.tile([D, 2], f32)
            ar_out = dp.tile([D, 2], f32)
            nc.sync.dma_start(out=ar_in[:], in_=stats[:])
            nc.gpsimd.collective_compute(
                "AllReduce",
                mybir.AluOpType.add,
                replica_groups=[list(range(N_CORES))],
                ins=[ar_in.opt()],
                outs=[ar_out.opt()],
            )
            arr = pp.tile([D, 2], f32, tag="arr")
            nc.sync.dma_start(out=arr[:], in_=ar_out[:])

            mean = pp.tile([D, 1], f32, tag="mean")
            var = pp.tile([D, 1], f32, tag="var")
            tmp = pp.tile([D, 1], f32, tag="tmp")
            A = pp.tile([D, 1], f32, tag="A")
            B = pp.tile([D, 1], f32, tag="B")
            epsT = pp.tile([D, 1], f32, tag="epsT")
            nc.vector.memset(epsT[:], EPS)
            nc.vector.tensor_scalar_mul(mean[:], arr[:, 0:1], inv_n)
            nc.vector.tensor_scalar_mul(var[:], arr[:, 1:2], inv_n)
            nc.vector.tensor_mul(tmp[:], mean[:], mean[:])
            nc.vector.tensor_sub(var[:], var[:], tmp[:])
            # std = sqrt(var + eps); inv = 1/std
            nc.scalar.activation(
                out=tmp[:], in_=var[:],
                func=mybir.ActivationFunctionType.Sqrt, bias=epsT[:],
            )
            nc.vector.reciprocal(var[:], tmp[:])  # var now holds 1/std
            nc.vector.tensor_mul(A[:], var[:], gam_sb[:])
            nc.vector.tensor_mul(tmp[:], mean[:], A[:])
            nc.vector.tensor_sub(B[:], bet_sb[:], tmp[:])

            # ---- phase D: relu(h2*A+B) + x, back to node-major layout
            for j0 in range(0, C, 8):
                j1 = min(j0 + 8, C)
                rT = wp.tile([D, 8 * P], f32, tag="rT")
                nc.scalar.activation(
                    out=rT[:, : (j1 - j0) * P], in_=h2T[:, j0 * P : j1 * P],
                    func=mybir.ActivationFunctionType.Relu,
                    scale=A[:], bias=B[:],
                )
                r_p = psp.tile([P, 8 * D], f32, tag="r_p")
                for j in range(j0, j1):
                    nc.tensor.transpose(
                        out=r_p[:, (j - j0) * D : (j - j0 + 1) * D],
                        in_=rT[:, (j - j0) * P : (j - j0 + 1) * P],
                        identity=ident[:D, :D],
                    )
                nc.vector.tensor_add(
                    agg[:, j0 * D : j1 * D], r_p[:, : (j1 - j0) * D],
                    x_sb[:, j0 * D : j1 * D],
                )
                if j1 % 16 == 0 or j1 == C:
                    ja = (j1 - 1) // 16 * 16
                    nc.sync.dma_start(
                        out=out_d[:, ja * D : j1 * D],
                        in_=agg[:, ja * D : j1 * D],
                    )

    nc.compile()
    return nc


# ----------------------------------------------------------------------------
# Entry point
# ----------------------------------------------------------------------------
def kernel(x, src, dst, W, b, gamma, beta):
    from concourse.bass_utils import run_bass_kernel_spmd

    x = np.asarray(x, dtype=np.float32)
    src = np.asarray(src, dtype=np.int32)
    dst = np.asarray(dst, dtype=np.int32)
    W = np.asarray(W, dtype=np.float32)
    gamma = np.asarray(gamma, dtype=np.float32)
    beta = np.asarray(beta, dtype=np.float32)
    n_nodes = x.shape[0]
    assert x.shape[1] == D

    plan = _build_plan(src, dst, n_nodes)
    wins = _build_windows(plan, n_nodes)
    idx_packed = _pack_idx(plan, wins)
    C, M, rows = plan["C"], plan["M"], plan["rows"]
    R, Rg, Wstart = wins["R"], wins["Rg"], wins["Wstart"]

    # gather tables: y = x * norm_src rows, windowed + deduped per group
    y = x * plan["ns"][:, None]

    in_maps = []
    for c in range(N_CORES):
        t_c = np.zeros((R, D), dtype=np.float32)
        for g in range(wins["n_groups"]):
            wn = wins["win_nodes"][c][g]
            s = int(Wstart[g])
            t_c[s + 1 : s + 1 + len(wn)] = y[wn]
        perm = plan["perms"][c]
        xp = np.zeros((rows, D), dtype=np.float32)
        real = perm >= 0
        xp[real] = x[perm[real]]
        # [rows, D] -> [P, C*D] with row j*128+p at (p, j*64:(j+1)*64)
        xp_dev = np.ascontiguousarray(
            xp.reshape(C, P, D).transpose(1, 0, 2).reshape(P, C * D)
        )
        in_maps.append(
            dict(
                t=t_c,
                idx=np.ascontiguousarray(idx_packed[c]),
                nd=np.ascontiguousarray(plan["nd_dev"][c]),
                xp=xp_dev,
                w=W,
                gam=gamma.reshape(D, 1),
                bet=beta.reshape(D, 1),
            )
        )

    nc = _build_bass(
        n_nodes, C, M, plan["K"], plan["off"], plan["groups"],
        plan["zero_buckets"], Rg, Wstart, R, n_nodes,
    )
    res = run_bass_kernel_spmd(nc, in_maps, core_ids=list(range(N_CORES)))
    kernel.last_results = res

    out = np.empty_like(x)
    for c in range(N_CORES):
        o = res.results[c]["out"]  # [P, C*D]
        o_rows = o.reshape(P, C, D).transpose(1, 0, 2).reshape(rows, D)
        perm = plan["perms"][c]
        real = perm >= 0
        out[perm[real]] = o_rows[real]
    return out
